# revision 24
# baseline (speedup 1.0000x reference)
"""Trainium2 Bass kernel for a single-head AttentionBlock with residual.

Reference computation (per batch b):
    q = x @ Wq^T ; k = x @ Wk^T ; v = x @ Wv^T        (bq/bk zero per spec)
    s = (q @ k^T) / sqrt(D)                            [S, S]
    s = where(mask[b] == 0 (keys), -1e10, s)
    a = softmax(s, axis=-1)
    out = x + (a @ v) @ Wo^T + (Wo bv + bo)

Algebraic restructure (exact):
  * scores = x_q @ (Wq^T Wk) @ x_k^T -- fold Wq into the K projection:
        ktil = x_k @ (Wk^T Wq)  =>  scores = x_q . ktil   (no Q projection)
  * (a @ v) @ Wo^T = a @ (x_k @ (Wo Wv)^T) -- fold Wo into the V projection:
        vtil = x_k @ (Wo Wv)^T  =>  out = x_q + a @ vtil  (no out projection)
  * masked keys contribute exactly 0 to softmax num/denom (exp(-1e10) == 0
    in fp32), so keys are host-compacted: only kept keys (mask==1) are
    shipped/projected, padded up to KT*128 with -30000-bias slots.

Sharding: 8 cores = 4 batches x 2 query-halves, no collectives. Each core
projects ktil/vtil for all kept keys of its batch (~1012-1044 here, padded
to KT=9 tiles / 1152 slots) and attends its 1024 queries. ~9.7 GFLOP/core
of matmul, ~2.7x less than the unfolded uncompacted algorithm. (A variant
that key-splits the projections across the batch pair and exchanges via
2-member AllGathers was tried and is ~37us SLOWER: the collectives core
needs ~40us+ from kernel start before an exchange completes, which a
~95us kernel cannot hide.)

All matmuls run in fp8 (e4m3, TRN flavor: max +-240) with
perf_mode=DoubleRow: 256-row virtual contraction at ~1 output element per
cycle (measured ~216ns per [128,512] matmul, ~1.87x over fp16). Weights are host-scaled x32 so their entries are ~N(0,1) in fp8;
the 1/32 is removed in the PSUM evictions. 1/sqrt(D) is applied as the exp
activation scale; exp is additionally scaled by 1/16 (bias -ln16) so the
fp8 expt tile stays in e4m3 range. The softmax denominator comes from
ones-vector DoubleRow matmuls accumulated over key tiles, transposed to
per-partition scalars with tiny fp32 matmuls, and applied together with
the residual add in one DVE scalar_tensor_tensor per output chunk.

Softmax max-subtraction is skipped: scores are ~N(0,1), exp < ~200 fits
fp32 and the /16 keeps expt in fp8 range.

nonzero bq/bk (spec says zeros) or an all-masked batch trigger an exact
numpy fallback.
"""

import functools
from contextlib import ExitStack

import ml_dtypes
import numpy as np

import concourse.bass as bass
import concourse.tile as tile
from concourse import bacc, mybir
from concourse.bass_utils import run_bass_kernel_spmd

P = 128
NEG_BIAS = -30000.0
N_CORES = 8
WSCALE = 32.0        # weight tensors stored x32 so entries are ~N(0,1) in fp8
EXP_SCALE = 16.0     # exp stored /16 so expt stays in e4m3 range
NP_FP8 = ml_dtypes.float8_e4m3  # TRN float8e4: max normal +-240


def _chunks(total, size):
    return [(o, min(size, total - o)) for o in range(0, total, size)]


def build_program(D=1024, SQ=1024, KT=9, n_cores=8, dedup=False):
    """Build + compile the single-core Bass program (same program on all cores).

    KT: number of 128-row key tiles (kept keys padded to KT*128).
    """
    f32 = mybir.dt.float32
    f16 = mybir.dt.float16
    fp8 = mybir.dt.float8e4
    DR = mybir.MatmulPerfMode.DoubleRow
    DT = D // P          # contraction tiles over d (and d' / e)
    QT = SQ // P         # query row tiles
    KPAD = KT * P
    assert DT % 2 == 0

    Exp = mybir.ActivationFunctionType.Exp
    mult = mybir.AluOpType.mult
    add = mybir.AluOpType.add

    nc = bacc.Bacc("TRN2", target_bir_lowering=False, debug=False,
                   num_devices=n_cores)

    xqt_d = nc.dram_tensor("xqt", [D, SQ], fp8, kind="ExternalInput")
    xkt_d = nc.dram_tensor("xkt", [D, KPAD], fp8, kind="ExternalInput")
    mt_d = nc.dram_tensor("mt", [D, D], fp8, kind="ExternalInput")   # (Wk^T Wq)*32
    wvo_d = nc.dram_tensor("wvo", [D, D], fp8, kind="ExternalInput")  # (Wo Wv)^T*32
    mb_d = nc.dram_tensor("mb", [P, KT], f32, kind="ExternalInput")
    hs_d = nc.dram_tensor("hs", [SQ, D], f16, kind="ExternalInput")
    out_d = nc.dram_tensor("out", [SQ, D], f32, kind="ExternalOutput")

    with tile.TileContext(nc) as tc, ExitStack() as ctx:
        sb = ctx.enter_context(tc.tile_pool(name="sb", bufs=1))
        outp = ctx.enter_context(tc.tile_pool(name="outs", bufs=2))
        con = ctx.enter_context(tc.tile_pool(name="const", bufs=1))
        pp = ctx.enter_context(tc.tile_pool(name="pp", bufs=5, space="PSUM"))
        rsp = ctx.enter_context(tc.tile_pool(name="rsp", bufs=1, space="PSUM"))

        # ---- PE warmup during the initial DMA wait (HAM ramp) ----
        ones1h = con.tile([1, 1], f16)
        nc.gpsimd.memset(ones1h[:], 1.0)
        warm_in = con.tile([1, 256], f16)
        nc.gpsimd.memset(warm_in[:], 0.0)
        warm_ps = pp.tile([P, 512], f32, tag="pp")
        N_WARM = 16
        for i in range(N_WARM):
            nc.tensor.matmul(warm_ps[:1, :256], ones1h[:], warm_in[:],
                             start=(i == 0), stop=(i == N_WARM - 1))
        warm_out = con.tile([1, 256], f32)
        nc.vector.tensor_copy(warm_out[:], warm_ps[:1, :256])

        # ---- constants ----
        mb = con.tile([P, KT], f32)
        nc.gpsimd.dma_start(mb[:], mb_d.ap())
        ones1 = con.tile([1, 1], f32)
        nc.gpsimd.memset(ones1[:], 1.0)
        ones128h = con.tile([P, 1], f16)
        nc.gpsimd.memset(ones128h[:], 1.0)

        # ---- DMA loads: first-needed first, split across the 3 queues ----
        _engs = [nc.gpsimd, nc.sync, nc.scalar]

        mt_sb = sb.tile([P, DT, D], fp8)
        xkt_sb = sb.tile([P, DT, KPAD], fp8)
        wvo_sb = sb.tile([P, DT, D], fp8)
        xqt_sb = sb.tile([P, DT, SQ], fp8)
        hs_sb = sb.tile([P, QT, D], f16)

        mt_v = mt_d.ap().rearrange("(t p) e -> p t e", p=P)
        xkt_v = xkt_d.ap().rearrange("(t p) k -> p t k", p=P)
        wvo_v = wvo_d.ap().rearrange("(t p) e -> p t e", p=P)
        xqt_v = xqt_d.ap().rearrange("(t p) q -> p t q", p=P)
        hs_v = hs_d.ap().rearrange("(t p) f -> p t f", p=P)

        ei = 0
        # mt + xkt per contraction pair, interleaved so the first k-proj
        # accumulation group can start as soon as pair 0 lands.
        # (column-block loads of xkt were tried to let the v-proj start
        # first on partial data: 384B-per-partition bursts run at ~55GB/s
        # vs ~120GB/s for these 1152B row-chunk bursts -- net loss.)
        for dp in range(DT // 2):
            sl = slice(2 * dp, 2 * dp + 2)
            _engs[ei % 3].dma_start(mt_sb[:, sl, :], mt_v[:, sl, :]); ei += 1
            _engs[ei % 3].dma_start(xkt_sb[:, sl, :], xkt_v[:, sl, :]); ei += 1
        for dp in range(DT // 2):
            sl = slice(2 * dp, 2 * dp + 2)
            _engs[ei % 3].dma_start(wvo_sb[:, sl, :], wvo_v[:, sl, :]); ei += 1
        for dp in range(DT // 2):
            sl = slice(2 * dp, 2 * dp + 2)
            _engs[ei % 3].dma_start(xqt_sb[:, sl, :], xqt_v[:, sl, :]); ei += 1
        for hh in range(2):
            sl = slice(hh * (QT // 2), (hh + 1) * (QT // 2))
            _engs[ei % 3].dma_start(hs_sb[:, sl, :], hs_v[:, sl, :]); ei += 1

        ktil = sb.tile([P, DT, KPAD], fp8)   # ktil^T: [d'-part, d'-tile, k]
        vtil = sb.tile([P, KT, D], fp8)      # vtil:   [k-part, k-tile, f]
        expt = sb.tile([P, KT, SQ], fp8)     # exp(scores)^T/16: [k-part, k-tile, q]

        # only DVE and ACT can read PSUM (GPSIMD cannot)
        _ev = [nc.vector, nc.scalar]
        evi = 0

        def evict(dst, src_ps):
            nonlocal evi
            e = _ev[evi % 2]
            evi += 1
            if e is nc.scalar:
                e.mul(dst, src_ps, 1.0 / WSCALE)
            else:
                e.tensor_scalar_mul(dst, src_ps, 1.0 / WSCALE)

        # ---- ktil = ((Wk^T Wq) @ x_k^T)  [d', k], DoubleRow over d ----
        kchunks = _chunks(KPAD, 512)
        for et in range(DT):
            pss = [pp.tile([P, 512], f32, tag="pp", name=f"ps_k{et}_{i}")
                   for i in range(len(kchunks))]
            for dp in range(DT // 2):
                lhsT = mt_sb[:, 2 * dp:2 * dp + 2, et * P:(et + 1) * P]
                for ci, (ko, kn) in enumerate(kchunks):
                    nc.tensor.matmul(
                        pss[ci][:, :kn], lhsT,
                        xkt_sb[:, 2 * dp:2 * dp + 2, ko:ko + kn],
                        start=(dp == 0), stop=(dp == DT // 2 - 1),
                        perf_mode=DR)
            for ci, (ko, kn) in enumerate(kchunks):
                evict(ktil[:, et, ko:ko + kn], pss[ci][:, :kn])

        # ---- vtil = x_k @ (Wo Wv)^T  [k, f], DoubleRow over d ----
        fchunks = _chunks(D, 512)
        for kt in range(KT):
            pss = [pp.tile([P, 512], f32, tag="pp", name=f"ps_v{kt}_{i}")
                   for i in range(len(fchunks))]
            for dp in range(DT // 2):
                lhsT = xkt_sb[:, 2 * dp:2 * dp + 2, kt * P:(kt + 1) * P]
                for ci, (fo, fn) in enumerate(fchunks):
                    nc.tensor.matmul(
                        pss[ci][:, :fn], lhsT,
                        wvo_sb[:, 2 * dp:2 * dp + 2, fo:fo + fn],
                        start=(dp == 0), stop=(dp == DT // 2 - 1),
                        perf_mode=DR)
            for ci, (fo, fn) in enumerate(fchunks):
                evict(vtil[:, kt, fo:fo + fn], pss[ci][:, :fn])

        # ---- scores^T + exp: expt = exp(s/sqrt(D) - ln16 + mb) ----
        # rs[1, q] += ones.T @ expt  (DoubleRow pairs of key tiles)
        qchunks = _chunks(SQ, 512)
        # fp16 partial row-sums accumulated on the otherwise-idle DVE
        # (the final partition reduction is then 2 fp16 matmuls)
        acc = con.tile([P, SQ], f16)
        nc.gpsimd.memset(acc[:], 0.0)
        for kt in range(KT):
            pss = [pp.tile([P, 512], f32, tag="pp", name=f"ps_s{kt}_{i}")
                   for i in range(len(qchunks))]
            for ep in range(DT // 2):
                lhsT = ktil[:, 2 * ep:2 * ep + 2, kt * P:(kt + 1) * P]
                for ci, (qo, qn) in enumerate(qchunks):
                    nc.tensor.matmul(
                        pss[ci][:, :qn], lhsT,
                        xqt_sb[:, 2 * ep:2 * ep + 2, qo:qo + qn],
                        start=(ep == 0), stop=(ep == DT // 2 - 1),
                        perf_mode=DR)
            for ci, (qo, qn) in enumerate(qchunks):
                nc.scalar.activation(
                    expt[:, kt, qo:qo + qn], pss[ci][:, :qn], Exp,
                    bias=mb[:, kt:kt + 1], scale=float(D) ** -0.5)
            for ci, (qo, qn) in enumerate(qchunks):
                nc.vector.tensor_add(acc[:, qo:qo + qn], acc[:, qo:qo + qn],
                                     expt[:, kt, qo:qo + qn])

        # ---- out[q, f] = (expt.T @ vtil) * rinv[q] + hs[q, f] ----
        out_v = out_d.ap().rearrange("(t p) f -> t p f", p=P)
        out_engs = [nc.sync, nc.scalar, nc.gpsimd]

        def av_mms(qt):
            pss = [pp.tile([P, 512], f32, tag="pp", name=f"ps_o{qt}_{i}")
                   for i in range(len(fchunks))]
            for ktp in range(KT // 2):
                lhsT = expt[:, 2 * ktp:2 * ktp + 2, qt * P:(qt + 1) * P]
                for ci, (fo, fn) in enumerate(fchunks):
                    nc.tensor.matmul(
                        pss[ci][:, :fn], lhsT,
                        vtil[:, 2 * ktp:2 * ktp + 2, fo:fo + fn],
                        start=(ktp == 0),
                        stop=(ktp == KT // 2 - 1 and KT % 2 == 0),
                        perf_mode=DR)
            if KT % 2 == 1:
                lhsT = expt[:, KT - 1, qt * P:(qt + 1) * P]
                for ci, (fo, fn) in enumerate(fchunks):
                    nc.tensor.matmul(
                        pss[ci][:, :fn], lhsT,
                        vtil[:, KT - 1, fo:fo + fn],
                        start=(KT == 1), stop=True)
            return pss

        # qt0's A@V matmuls are emitted first so the rsum reduce chain
        # (DVE adds -> ones-matmul -> copy -> transpose -> reciprocal)
        # hides behind ~2us of PE work instead of stalling the PE queue
        pss0 = av_mms(0)

        # ---- 1/rsum as per-partition scalars: [1, SQ] -> [P, QT] ----
        rss = [rsp.tile([1, 512], f32, tag=f"rs{ci}", name=f"rs{ci}")
               for ci in range(len(qchunks))]
        rsum_sb = con.tile([1, SQ], f32)
        for ci, (qo, qn) in enumerate(qchunks):
            nc.tensor.matmul(rss[ci][:, :qn], ones128h[:], acc[:, qo:qo + qn],
                             start=True, stop=True)
            nc.scalar.copy(rsum_sb[:, qo:qo + qn], rss[ci][:, :qn])
        rsT = rsp.tile([P, QT], f32, tag="rsT")
        for t in range(QT):
            nc.tensor.matmul(
                rsT[:, t:t + 1], rsum_sb[:, t * P:(t + 1) * P], ones1[:],
                start=(t == 0), stop=(t == QT - 1))
        rinv = con.tile([P, QT], f32)
        nc.vector.reciprocal(rinv[:], rsT[:])

        for qt in range(QT):
            pss = pss0 if qt == 0 else av_mms(qt)
            outt = outp.tile([P, D], f32, tag="outt", name=f"outt{qt}")
            for ci, (fo, fn) in enumerate(fchunks):
                nc.vector.scalar_tensor_tensor(
                    outt[:, fo:fo + fn], pss[ci][:, :fn], rinv[:, qt:qt + 1],
                    hs_sb[:, qt, fo:fo + fn], op0=mult, op1=add)
                if qt >= QT - 2 and fn > 256:
                    # tail tiles: halve each chunk so the final bytes drain
                    # on more queues in parallel
                    h2 = fn // 2
                    out_engs[(qt * 4 + 2 * ci) % 3].dma_start(
                        out_v[qt][:, fo:fo + h2], outt[:, fo:fo + h2])
                    out_engs[(qt * 4 + 2 * ci + 1) % 3].dma_start(
                        out_v[qt][:, fo + h2:fo + fn], outt[:, fo + h2:fo + fn])
                else:
                    out_engs[(qt * 2 + ci) % 3].dma_start(
                        out_v[qt][:, fo:fo + fn], outt[:, fo:fo + fn])

    nc.compile()
    return nc


DEDUP = False


@functools.lru_cache(maxsize=4)
def _get_program(D, SQ, KT, dedup=DEDUP):
    return build_program(D, SQ, KT, dedup=dedup)


def _numpy_reference(hidden_states, mask, Wq, bq, Wk, bk, Wv, bv, Wo, bo):
    """Exact fallback (used only for inputs outside the spec envelope)."""
    x = hidden_states.astype(np.float64)
    q = x @ Wq.T.astype(np.float64) + bq
    k = x @ Wk.T.astype(np.float64) + bk
    v = x @ Wv.T.astype(np.float64) + bv
    s = np.einsum("bqd,bkd->bqk", q, k) / np.sqrt(x.shape[-1])
    s = np.where(mask[:, None, :] == 0, -1e10, s)
    s -= s.max(axis=-1, keepdims=True)
    e = np.exp(s)
    a = e / e.sum(axis=-1, keepdims=True)
    hid = np.einsum("bqk,bkd->bqd", a, v)
    out = x + hid @ Wo.T.astype(np.float64) + bo
    return out.astype(np.float32)


def _fp8(a):
    return np.ascontiguousarray(
        np.clip(a, -240.0, 240.0).astype(NP_FP8))


def pick_kt(mask, dedup=DEDUP):
    nb = (np.asarray(mask) != 0).sum(axis=1)
    if dedup:
        # per-member local slots must cover ceil(nb/2); gathered = 2 halves
        kth = (int(nb.max() + 1) // 2 + P - 1) // P
        return 2 * kth, nb
    return (int(nb.max()) + P - 1) // P, nb


def make_in_maps(hidden_states, mask, Wq, bq, Wk, bk, Wv, bv, Wo, bo, KT,
                 dedup=DEDUP):
    hs = np.asarray(hidden_states, dtype=np.float32)
    mask = np.asarray(mask)
    B, S, D = hs.shape
    SQ = S // 2
    KPAD = KT * P

    Wq64 = np.asarray(Wq, np.float64)
    Wk64 = np.asarray(Wk, np.float64)
    Wv64 = np.asarray(Wv, np.float64)
    Wo64 = np.asarray(Wo, np.float64)
    # scores = x_q @ (Wq^T Wk) @ x_k^T ; ktil-proj lhsT[d, d'] = (Wk^T Wq)[d, d']
    mt_h = _fp8(Wk64.T @ Wq64 * WSCALE)
    # out = a @ (x_k @ (Wo Wv)^T) ; vtil-proj rhs[d, f] = (Wo Wv)^T[d, f]
    wvo_h = _fp8((Wo64 @ Wv64).T * WSCALE)
    # v-bias and o-bias act as a constant shift after the output projection
    extra = (np.asarray(Wo, np.float32) @ np.asarray(bv, np.float32)
             + np.asarray(bo, np.float32))

    in_maps = []
    for c in range(N_CORES):
        b, h = divmod(c, 2)
        xb = hs[b]
        keep = np.nonzero(mask[b] != 0)[0]
        nb = len(keep)
        if dedup:
            # member h projects half the kept keys; gathered key space is
            # [member0 block | member1 block], KLOC slots each
            KLOC = KPAD // 2
            n0 = (nb + 1) // 2
            loc = keep[:n0] if h == 0 else keep[n0:]
            xk = np.zeros((KLOC, D), np.float32)
            xk[:len(loc)] = xb[loc]
            mbv = np.full(KPAD, NEG_BIAS, np.float32)
            mbv[:n0] = -np.log(EXP_SCALE)
            mbv[KLOC:KLOC + nb - n0] = -np.log(EXP_SCALE)
        else:
            xk = np.zeros((KPAD, D), np.float32)
            xk[:nb] = xb[keep]
            mbv = np.full(KPAD, NEG_BIAS, np.float32)
            mbv[:nb] = -np.log(EXP_SCALE)
        xq = xb[h * SQ:(h + 1) * SQ]
        m = dict(
            xqt=_fp8(xq.T),
            xkt=_fp8(xk.T),
            mt=mt_h, wvo=wvo_h,
            mb=np.ascontiguousarray(mbv.reshape(KT, P).T),
            hs=np.ascontiguousarray((xq + extra[None, :]).astype(np.float16)),
        )
        in_maps.append(m)
    return in_maps


def assemble_output(results, B, S, D):
    SQ = S // 2
    out = np.empty((B, S, D), np.float32)
    for c in range(N_CORES):
        b, h = divmod(c, 2)
        out[b, h * SQ:(h + 1) * SQ, :] = results[c]["out"]
    return out


def kernel(hidden_states, mask, Wq, bq, Wk, bk, Wv, bv, Wo, bo):
    hs = np.asarray(hidden_states, dtype=np.float32)
    B, S, D = hs.shape
    args = dict(hidden_states=hs, mask=np.asarray(mask),
                Wq=np.asarray(Wq, np.float32), bq=np.asarray(bq, np.float32),
                Wk=np.asarray(Wk, np.float32), bk=np.asarray(bk, np.float32),
                Wv=np.asarray(Wv, np.float32), bv=np.asarray(bv, np.float32),
                Wo=np.asarray(Wo, np.float32), bo=np.asarray(bo, np.float32))
    KT, nb = pick_kt(args["mask"])
    if (np.any(args["bq"]) or np.any(args["bk"]) or nb.min() == 0
            or B * 2 != N_CORES or S % 256 or D % 256 or D < 512):
        return _numpy_reference(**args)

    nc = _get_program(D, S // 2, KT)
    in_maps = make_in_maps(**args, KT=KT)
    res = run_bass_kernel_spmd(nc, in_maps, core_ids=list(range(N_CORES)))
    return assemble_output(res.results, B, S, D)


if __name__ == "__main__":
    rng = np.random.default_rng(0)
    B, S, D = 4, 2048, 1024
    ins = dict(
        hidden_states=rng.standard_normal((B, S, D), np.float32),
        mask=rng.integers(0, 2, (B, S)).astype(np.int32),
        Wq=rng.standard_normal((D, D), np.float32) / np.sqrt(D),
        bq=np.zeros(D, np.float32),
        Wk=rng.standard_normal((D, D), np.float32) / np.sqrt(D),
        bk=np.zeros(D, np.float32),
        Wv=rng.standard_normal((D, D), np.float32) / np.sqrt(D),
        bv=np.zeros(D, np.float32),
        Wo=rng.standard_normal((D, D), np.float32) / np.sqrt(D),
        bo=np.zeros(D, np.float32),
    )
    out = kernel(**ins)
    ref = _numpy_reference(**ins)
    err = np.max(np.abs(out - ref)) / np.max(np.abs(ref))
    print("rel err vs numpy:", err)
